# revision 21
# baseline (speedup 1.0000x reference)
"""GCN layer (gather -> segment-sum -> linear) on 8 TRN2 NeuronCores.

out = segment_sum(x[src], dst, N) @ W.T + b

Strategy (dst-sharded SPMD, dense-adjacency matmul):
- The message-passing step is h = A @ x with A[d, s] = #edges s->d.
  Equivalently h^T = x^T @ A^T: contract over source nodes on TensorE.
- 10000 destinations split 1250 per core. Per core the host builds the
  dense fp8e4 adjacency block [10112 src, 1250 dst] (edge counts are small
  ints, exact in fp8e4) packed in the exact tile order the device streams:
  3 PSUM windows (512/512/226 dst) x 79 source slabs of 128, so every DMA
  is a contiguous line-rate read.
- Device: x lives in SBUF (pre-slabbed [128, 79*128] bf16, chunked loads
  interleaved with the adjacency stream on one HWDGE FIFO); per window, 79
  matmuls accumulate h^T[fi, dst] into a PSUM bank (lhsT = x slab
  [128 src, 128 fi] stationary bf16, rhs = A tile [128 src, ww] fp8);
  then h^T -> SBUF bf16, out^T = W @ h^T (bf16 matmul) + bias, DMA out.
- Host concatenates the per-core [128, 1250] outputs and transposes.
"""

import numpy as np
import ml_dtypes

N_NODES = 10000
D = 128
NCORES = 8
NPC = 1250  # dst nodes per core
WINDOWS = [(0, 512), (512, 512), (1024, 226)]  # (start, width) within a core
NSLAB = (N_NODES + 127) // 128  # 79 source slabs
NPAD = NSLAB * 128  # 10112
XGRP = 8  # x slabs per load chunk


def _groups(wi):
    """A-tile DMA group sizes (slabs per DMA) for window wi; ramped at the
    start of window 0 so the first matmul's data lands early."""
    ramp = [2, 4, 8] if wi == 0 else []
    left = NSLAB - sum(ramp)
    out = list(ramp)
    while left > 0:
        g = min(16, left)
        out.append(g)
        left -= g
    return out


def build_tables(src, dst, x, W, b):
    """Per-core packed adjacency + pre-slabbed x and other device inputs."""
    src = np.asarray(src).astype(np.int64)
    dst = np.asarray(dst).astype(np.int64)

    cnt = np.zeros((NPAD, NCORES * NPC), np.uint16)
    np.add.at(cnt, (src, dst), 1)
    adt = ml_dtypes.float8_e4m3 if cnt.max() <= 15 else ml_dtypes.bfloat16

    xs_pad = np.zeros((NPAD, D), np.float32)
    xs_pad[:N_NODES] = np.asarray(x, np.float32)
    # xs[p, s*128 + f] = x[s*128 + p, f]
    xs = np.ascontiguousarray(
        xs_pad.reshape(NSLAB, 128, D).transpose(1, 0, 2).reshape(128, NSLAB * D)
    ).astype(ml_dtypes.bfloat16)

    adjs = []
    for c in range(NCORES):
        ac = cnt[:, c * NPC : (c + 1) * NPC].astype(adt)
        ac = ac.reshape(NSLAB, 128, NPC)
        parts = []
        for wi, (ws, ww) in enumerate(WINDOWS):
            s0 = 0
            for ng in _groups(wi):
                # per-partition contiguous: [p][g][w]
                blk = ac[s0 : s0 + ng, :, ws : ws + ww].transpose(1, 0, 2)
                parts.append(np.ascontiguousarray(blk).reshape(-1))
                s0 += ng
        adjs.append(np.concatenate(parts))

    wt = np.asarray(W, np.float32).T.astype(ml_dtypes.bfloat16)  # [fi, fo]
    bb = np.asarray(b, np.float32).reshape(128, 1)
    return xs, adjs, wt, bb, adt


def build_program(adt=None):
    import concourse.mybir as mybir
    import concourse.tile as tile
    from concourse import bacc

    nc = bacc.Bacc("TRN2", target_bir_lowering=False, num_devices=NCORES)
    bf16 = mybir.dt.bfloat16
    f32 = mybir.dt.float32
    import ml_dtypes as _mld
    fp8 = (
        mybir.dt.bfloat16
        if adt is _mld.bfloat16
        else mybir.dt.float8e4
    )

    adj_len = sum(128 * ww for _, ww in WINDOWS) * NSLAB
    nchunk = (NSLAB + XGRP - 1) // XGRP
    xs_d = nc.declare_dram_parameter("xs", [128, NSLAB * D], bf16, isOutput=False)
    adj_d = nc.declare_dram_parameter("adj", [adj_len], fp8, isOutput=False)
    wt_d = nc.declare_dram_parameter("wt", [128, 128], bf16, isOutput=False)
    b_d = nc.declare_dram_parameter("bias", [128, 1], f32, isOutput=False)
    out_d = nc.declare_dram_parameter("out", [128, NPC], f32, isOutput=True)

    with tile.TileContext(nc) as tc:
        with (
            tc.tile_pool(name="const", bufs=1) as const_pool,
            tc.tile_pool(name="adj", bufs=6) as adj_pool,
            tc.tile_pool(name="hbuf", bufs=2) as hbuf_pool,
            tc.tile_pool(name="obuf", bufs=2) as obuf_pool,
            tc.tile_pool(name="ph", bufs=2, space="PSUM") as ph_pool,
            tc.tile_pool(name="po", bufs=2, space="PSUM") as po_pool,
        ):
            # x chunks: interleaved with the A stream on the same HWDGE FIFO
            # so the first matmul's inputs land first and later chunks pace in.
            xsc = [
                const_pool.tile([128, XGRP * D], bf16, tag=f"xs{k}", name=f"xs{k}")
                for k in range(nchunk)
            ]

            def load_chunk(k):
                lo, hi = k * XGRP, min((k + 1) * XGRP, NSLAB)
                nc.sync.dma_start(
                    xsc[k][:, 0 : (hi - lo) * D], xs_d[:, lo * D : hi * D]
                )

            wt_sb = const_pool.tile([128, 128], bf16)
            nc.scalar.dma_start(wt_sb[:], wt_d[:])
            b_sb = const_pool.tile([128, 1], f32)
            nc.scalar.dma_start(b_sb[:], b_d[:])

            load_chunk(0)
            next_chunk = 1
            off = 0
            for wi, (ws, ww) in enumerate(WINDOWS):
                ph = ph_pool.tile([128, ww], f32)
                s = 0
                for ng in _groups(wi):
                    at = adj_pool.tile([128, 16, ww], fp8, tag="adj")
                    nc.sync.dma_start(
                        at[:, 0:ng, :],
                        adj_d[off : off + ng * 128 * ww].rearrange(
                            "(p g w) -> p g w", g=ng, w=ww
                        ),
                    )
                    off += ng * 128 * ww
                    # everything the upcoming matmuls read must be issued first
                    while next_chunk < nchunk and next_chunk * XGRP < s + ng:
                        load_chunk(next_chunk)
                        next_chunk += 1
                    for g in range(ng):
                        nc.tensor.matmul(
                            ph[:, :],
                            lhsT=xsc[(s + g) // XGRP][
                                :, ((s + g) % XGRP) * D : ((s + g) % XGRP + 1) * D
                            ],
                            rhs=at[:, g, :],
                            start=(s + g == 0),
                            stop=(s + g == NSLAB - 1),
                        )
                    s += ng
                hb = hbuf_pool.tile([128, ww], bf16)
                nc.vector.tensor_copy(hb[:], ph[:, :])
                po = po_pool.tile([128, ww], f32)
                nc.tensor.matmul(po[:, :], lhsT=wt_sb[:], rhs=hb[:], start=True, stop=True)
                ob = obuf_pool.tile([128, ww], f32)
                nc.vector.tensor_scalar_add(ob[:], po[:, :], b_sb[:, 0:1])
                nc.scalar.dma_start(out_d[:, ws : ws + ww], ob[:])

    nc.compile()
    return nc


_CACHED_NC = {}


def kernel(x, src, dst, W, b):
    from concourse.bass_utils import run_bass_kernel_spmd

    xs, adjs, wt, bb, adt = build_tables(src, dst, x, W, b)
    key = np.dtype(adt).name
    if key not in _CACHED_NC:
        _CACHED_NC[key] = build_program(adt)
    nc = _CACHED_NC[key]

    in_maps = [
        {"xs": xs, "adj": adjs[c], "wt": wt, "bias": bb} for c in range(NCORES)
    ]
    res = run_bass_kernel_spmd(nc, in_maps, list(range(NCORES)))
    out_t = np.concatenate([res.results[c]["out"] for c in range(NCORES)], axis=1)
    return np.ascontiguousarray(out_t.T[:N_NODES]).astype(np.float32)


# revision 22
# speedup vs baseline: 1.0350x; 1.0350x over previous
"""GCN layer (gather -> segment-sum -> linear) on 8 TRN2 NeuronCores.

out = segment_sum(x[src], dst, N) @ W.T + b

Strategy (dst-sharded SPMD, dense-adjacency matmul):
- The message-passing step is h = A @ x with A[d, s] = #edges s->d.
  Equivalently h^T = x^T @ A^T: contract over source nodes on TensorE.
- 10000 destinations split 1250 per core. Per core the host builds the
  dense fp8e4 adjacency block [10112 src, 1250 dst] (edge counts are small
  ints, exact in fp8e4) packed in the exact tile order the device streams:
  3 PSUM windows (512/512/226 dst) x 79 source slabs of 128, so every DMA
  is a contiguous line-rate read.
- Device: x lives in SBUF (pre-slabbed [128, 79*128] bf16, chunked loads
  interleaved with the adjacency stream on one HWDGE FIFO); per window, 79
  matmuls accumulate h^T[fi, dst] into a PSUM bank (lhsT = x slab
  [128 src, 128 fi] stationary bf16, rhs = A tile [128 src, ww] fp8);
  then h^T -> SBUF bf16, out^T = W @ h^T (bf16 matmul) + bias, DMA out.
- Host concatenates the per-core [128, 1250] outputs and transposes.
"""

import numpy as np
import ml_dtypes

N_NODES = 10000
D = 128
NCORES = 8
NPC = 1250  # dst nodes per core
WINDOWS = [(0, 512), (512, 512), (1024, 226)]  # (start, width) within a core
NSLAB = (N_NODES + 127) // 128  # 79 source slabs
NPAD = NSLAB * 128  # 10112
XGRP = 8  # x slabs per load chunk


def _groups(wi):
    """A-tile DMA group sizes (slabs per DMA) for window wi; ramped at the
    start of window 0 so the first matmul's data lands early."""
    ramp = [2, 4, 8] if wi == 0 else []
    left = NSLAB - sum(ramp)
    out = list(ramp)
    while left > 0:
        g = min(16, left)
        out.append(g)
        left -= g
    return out


def build_tables(src, dst, x, W, b):
    """Per-core packed adjacency + pre-slabbed x and other device inputs."""
    src = np.asarray(src).astype(np.int64)
    dst = np.asarray(dst).astype(np.int64)

    cnt = np.zeros((NPAD, NCORES * NPC), np.uint16)
    np.add.at(cnt, (src, dst), 1)
    adt = ml_dtypes.float8_e4m3 if cnt.max() <= 15 else ml_dtypes.bfloat16

    xs_pad = np.zeros((NPAD, D), np.float32)
    xs_pad[:N_NODES] = np.asarray(x, np.float32)
    # xs[p, s*128 + f] = x[s*128 + p, f]
    xs = np.ascontiguousarray(
        xs_pad.reshape(NSLAB, 128, D).transpose(1, 0, 2).reshape(128, NSLAB * D)
    ).astype(ml_dtypes.bfloat16)

    adjs = []
    for c in range(NCORES):
        ac = cnt[:, c * NPC : (c + 1) * NPC].astype(adt)
        ac = ac.reshape(NSLAB, 128, NPC)
        parts = []
        for wi, (ws, ww) in enumerate(WINDOWS):
            s0 = 0
            for ng in _groups(wi):
                # per-partition contiguous: [p][g][w]
                blk = ac[s0 : s0 + ng, :, ws : ws + ww].transpose(1, 0, 2)
                parts.append(np.ascontiguousarray(blk).reshape(-1))
                s0 += ng
        adjs.append(np.concatenate(parts))

    wt = np.asarray(W, np.float32).T.astype(ml_dtypes.bfloat16)  # [fi, fo]
    bb = np.asarray(b, np.float32).reshape(128, 1)
    return xs, adjs, wt, bb, adt


def build_program(adt=None):
    import concourse.mybir as mybir
    import concourse.tile as tile
    from concourse import bacc

    nc = bacc.Bacc("TRN2", target_bir_lowering=False, num_devices=NCORES)
    bf16 = mybir.dt.bfloat16
    f32 = mybir.dt.float32
    import ml_dtypes as _mld
    fp8 = (
        mybir.dt.bfloat16
        if adt is _mld.bfloat16
        else mybir.dt.float8e4
    )

    adj_len = sum(128 * ww for _, ww in WINDOWS) * NSLAB
    nchunk = (NSLAB + XGRP - 1) // XGRP
    xs_d = nc.declare_dram_parameter("xs", [128, NSLAB * D], bf16, isOutput=False)
    adj_d = nc.declare_dram_parameter("adj", [adj_len], fp8, isOutput=False)
    wt_d = nc.declare_dram_parameter("wt", [128, 128], bf16, isOutput=False)
    b_d = nc.declare_dram_parameter("bias", [128, 1], f32, isOutput=False)
    out_d = nc.declare_dram_parameter("out", [128, NPC], f32, isOutput=True)

    with tile.TileContext(nc) as tc:
        with (
            tc.tile_pool(name="const", bufs=1) as const_pool,
            tc.tile_pool(name="adj", bufs=6) as adj_pool,
            tc.tile_pool(name="hbuf", bufs=2) as hbuf_pool,
            tc.tile_pool(name="obuf", bufs=2) as obuf_pool,
            tc.tile_pool(name="ph", bufs=2, space="PSUM") as ph_pool,
            tc.tile_pool(name="po", bufs=2, space="PSUM") as po_pool,
        ):
            # x chunks: interleaved with the A stream on the same HWDGE FIFO
            # so the first matmul's inputs land first and later chunks pace in.
            xsc = [
                const_pool.tile([128, XGRP * D], bf16, tag=f"xs{k}", name=f"xs{k}")
                for k in range(nchunk)
            ]

            def load_chunk(k):
                lo, hi = k * XGRP, min((k + 1) * XGRP, NSLAB)
                nc.scalar.dma_start(
                    xsc[k][:, 0 : (hi - lo) * D], xs_d[:, lo * D : hi * D]
                )

            wt_sb = const_pool.tile([128, 128], bf16)
            nc.scalar.dma_start(wt_sb[:], wt_d[:])
            b_sb = const_pool.tile([128, 1], f32)
            nc.scalar.dma_start(b_sb[:], b_d[:])

            load_chunk(0)
            next_chunk = 1
            off = 0
            for wi, (ws, ww) in enumerate(WINDOWS):
                ph = ph_pool.tile([128, ww], f32)
                s = 0
                for ng in _groups(wi):
                    at = adj_pool.tile([128, 16, ww], fp8, tag="adj")
                    nc.sync.dma_start(
                        at[:, 0:ng, :],
                        adj_d[off : off + ng * 128 * ww].rearrange(
                            "(p g w) -> p g w", g=ng, w=ww
                        ),
                    )
                    off += ng * 128 * ww
                    # everything the upcoming matmuls read must be issued first
                    while next_chunk < nchunk and next_chunk * XGRP < s + ng:
                        load_chunk(next_chunk)
                        next_chunk += 1
                    for g in range(ng):
                        nc.tensor.matmul(
                            ph[:, :],
                            lhsT=xsc[(s + g) // XGRP][
                                :, ((s + g) % XGRP) * D : ((s + g) % XGRP + 1) * D
                            ],
                            rhs=at[:, g, :],
                            start=(s + g == 0),
                            stop=(s + g == NSLAB - 1),
                        )
                    s += ng
                hb = hbuf_pool.tile([128, ww], bf16)
                nc.vector.tensor_copy(hb[:], ph[:, :])
                po = po_pool.tile([128, ww], f32)
                nc.tensor.matmul(po[:, :], lhsT=wt_sb[:], rhs=hb[:], start=True, stop=True)
                ob = obuf_pool.tile([128, ww], f32)
                nc.vector.tensor_scalar_add(ob[:], po[:, :], b_sb[:, 0:1])
                nc.scalar.dma_start(out_d[:, ws : ws + ww], ob[:])

    nc.compile()
    return nc


_CACHED_NC = {}


def kernel(x, src, dst, W, b):
    from concourse.bass_utils import run_bass_kernel_spmd

    xs, adjs, wt, bb, adt = build_tables(src, dst, x, W, b)
    key = np.dtype(adt).name
    if key not in _CACHED_NC:
        _CACHED_NC[key] = build_program(adt)
    nc = _CACHED_NC[key]

    in_maps = [
        {"xs": xs, "adj": adjs[c], "wt": wt, "bias": bb} for c in range(NCORES)
    ]
    res = run_bass_kernel_spmd(nc, in_maps, list(range(NCORES)))
    out_t = np.concatenate([res.results[c]["out"] for c in range(NCORES)], axis=1)
    return np.ascontiguousarray(out_t.T[:N_NODES]).astype(np.float32)


# revision 23
# speedup vs baseline: 1.0396x; 1.0044x over previous
"""GCN layer (gather -> segment-sum -> linear) on 8 TRN2 NeuronCores.

out = segment_sum(x[src], dst, N) @ W.T + b

Strategy (dst-sharded SPMD, dense-adjacency matmul):
- The message-passing step is h = A @ x with A[d, s] = #edges s->d.
  Equivalently h^T = x^T @ A^T: contract over source nodes on TensorE.
- 10000 destinations split 1250 per core. Per core the host builds the
  dense fp8e4 adjacency block [10112 src, 1250 dst] (edge counts are small
  ints, exact in fp8e4) packed in the exact tile order the device streams:
  3 PSUM windows (512/512/226 dst) x 79 source slabs of 128, so every DMA
  is a contiguous line-rate read.
- Device: x lives in SBUF (pre-slabbed [128, 79*128] bf16, chunked loads
  interleaved with the adjacency stream on one HWDGE FIFO); per window, 79
  matmuls accumulate h^T[fi, dst] into a PSUM bank (lhsT = x slab
  [128 src, 128 fi] stationary bf16, rhs = A tile [128 src, ww] fp8);
  then h^T -> SBUF bf16, out^T = W @ h^T (bf16 matmul) + bias, DMA out.
- Host concatenates the per-core [128, 1250] outputs and transposes.
"""

import numpy as np
import ml_dtypes

N_NODES = 10000
D = 128
NCORES = 8
NPC = 1250  # dst nodes per core
WINDOWS = [(0, 512), (512, 512), (1024, 226)]  # (start, width) within a core
NSLAB = (N_NODES + 127) // 128  # 79 source slabs
NPAD = NSLAB * 128  # 10112
XGRP = 8  # x slabs per load chunk


def _groups(wi):
    """A-tile DMA group sizes (slabs per DMA) for window wi; ramped at the
    start of window 0 so the first matmul's data lands early."""
    ramp = [2, 4, 8] if wi == 0 else []
    left = NSLAB - sum(ramp)
    out = list(ramp)
    while left > 0:
        g = min(16, left)
        out.append(g)
        left -= g
    return out


def build_tables(src, dst, x, W, b):
    """Per-core packed adjacency + pre-slabbed x and other device inputs."""
    src = np.asarray(src).astype(np.int64)
    dst = np.asarray(dst).astype(np.int64)

    cnt = np.zeros((NPAD, NCORES * NPC), np.uint16)
    np.add.at(cnt, (src, dst), 1)
    adt = ml_dtypes.float8_e4m3 if cnt.max() <= 15 else ml_dtypes.bfloat16

    xs_pad = np.zeros((NPAD, D), np.float32)
    xs_pad[:N_NODES] = np.asarray(x, np.float32)
    # xs[p, s*128 + f] = x[s*128 + p, f]
    xs = np.ascontiguousarray(
        xs_pad.reshape(NSLAB, 128, D).transpose(1, 0, 2).reshape(128, NSLAB * D)
    ).astype(ml_dtypes.bfloat16)

    adjs = []
    for c in range(NCORES):
        ac = cnt[:, c * NPC : (c + 1) * NPC].astype(adt)
        ac = ac.reshape(NSLAB, 128, NPC)
        parts = []
        for wi, (ws, ww) in enumerate(WINDOWS):
            s0 = 0
            for ng in _groups(wi):
                # per-partition contiguous: [p][g][w]
                blk = ac[s0 : s0 + ng, :, ws : ws + ww].transpose(1, 0, 2)
                parts.append(np.ascontiguousarray(blk).reshape(-1))
                s0 += ng
        adjs.append(np.concatenate(parts))

    wt = np.asarray(W, np.float32).T.astype(ml_dtypes.bfloat16)  # [fi, fo]
    bb = np.asarray(b, np.float32).reshape(128, 1)
    return xs, adjs, wt, bb, adt


def build_program(adt=None):
    import concourse.mybir as mybir
    import concourse.tile as tile
    from concourse import bacc

    nc = bacc.Bacc("TRN2", target_bir_lowering=False, num_devices=NCORES)
    bf16 = mybir.dt.bfloat16
    f32 = mybir.dt.float32
    import ml_dtypes as _mld
    fp8 = (
        mybir.dt.bfloat16
        if adt is _mld.bfloat16
        else mybir.dt.float8e4
    )

    adj_len = sum(128 * ww for _, ww in WINDOWS) * NSLAB
    nchunk = (NSLAB + XGRP - 1) // XGRP
    xs_d = nc.declare_dram_parameter("xs", [128, NSLAB * D], bf16, isOutput=False)
    adj_d = nc.declare_dram_parameter("adj", [adj_len], fp8, isOutput=False)
    wt_d = nc.declare_dram_parameter("wt", [128, 128], bf16, isOutput=False)
    b_d = nc.declare_dram_parameter("bias", [128, 1], f32, isOutput=False)
    out_d = nc.declare_dram_parameter("out", [128, NPC], f32, isOutput=True)

    with tile.TileContext(nc) as tc:
        with (
            tc.tile_pool(name="const", bufs=1) as const_pool,
            tc.tile_pool(name="adj", bufs=6) as adj_pool,
            tc.tile_pool(name="hbuf", bufs=2) as hbuf_pool,
            tc.tile_pool(name="obuf", bufs=2) as obuf_pool,
            tc.tile_pool(name="ph", bufs=2, space="PSUM") as ph_pool,
            tc.tile_pool(name="po", bufs=2, space="PSUM") as po_pool,
        ):
            # x chunks: interleaved with the A stream on the same HWDGE FIFO
            # so the first matmul's inputs land first and later chunks pace in.
            xsc = [
                const_pool.tile([128, XGRP * D], bf16, tag=f"xs{k}", name=f"xs{k}")
                for k in range(nchunk)
            ]

            def load_chunk(k):
                lo, hi = k * XGRP, min((k + 1) * XGRP, NSLAB)
                nc.sync.dma_start(
                    xsc[k][:, 0 : (hi - lo) * D], xs_d[:, lo * D : hi * D]
                )

            wt_sb = const_pool.tile([128, 128], bf16)
            nc.scalar.dma_start(wt_sb[:], wt_d[:])
            b_sb = const_pool.tile([128, 1], f32)
            nc.scalar.dma_start(b_sb[:], b_d[:])

            load_chunk(0)
            next_chunk = 1
            off = 0
            for wi, (ws, ww) in enumerate(WINDOWS):
                ph = ph_pool.tile([128, ww], f32)
                s = 0
                for ng in _groups(wi):
                    at = adj_pool.tile([128, 16, ww], fp8, tag="adj")
                    nc.sync.dma_start(
                        at[:, 0:ng, :],
                        adj_d[off : off + ng * 128 * ww].rearrange(
                            "(p g w) -> p g w", g=ng, w=ww
                        ),
                    )
                    off += ng * 128 * ww
                    # everything the upcoming matmuls read must be issued first
                    while next_chunk < nchunk and next_chunk * XGRP < s + ng:
                        load_chunk(next_chunk)
                        next_chunk += 1
                    for g in range(ng):
                        nc.tensor.matmul(
                            ph[:, :],
                            lhsT=xsc[(s + g) // XGRP][
                                :, ((s + g) % XGRP) * D : ((s + g) % XGRP + 1) * D
                            ],
                            rhs=at[:, g, :],
                            start=(s + g == 0),
                            stop=(s + g == NSLAB - 1),
                        )
                    s += ng
                hb = hbuf_pool.tile([128, ww], bf16)
                nc.vector.tensor_copy(hb[:], ph[:, :])
                po = po_pool.tile([128, ww], f32)
                nc.tensor.matmul(po[:, :], lhsT=wt_sb[:], rhs=hb[:], start=True, stop=True)
                ob = obuf_pool.tile([128, ww], f32)
                nc.vector.tensor_scalar_add(ob[:], po[:, :], b_sb[:, 0:1])
                nc.scalar.dma_start(out_d[:, ws : ws + ww], ob[:])

    nc.compile()
    return nc


_CACHED_NC = {}


def kernel(x, src, dst, W, b):
    from concourse.bass_utils import run_bass_kernel_spmd

    xs, adjs, wt, bb, adt = build_tables(src, dst, x, W, b)
    key = np.dtype(adt).name
    if key not in _CACHED_NC:
        _CACHED_NC[key] = build_program(adt)
    nc = _CACHED_NC[key]

    in_maps = [
        {"xs": xs, "adj": adjs[c], "wt": wt, "bias": bb} for c in range(NCORES)
    ]
    res = run_bass_kernel_spmd(nc, in_maps, list(range(NCORES)))
    out_t = np.concatenate([res.results[c]["out"] for c in range(NCORES)], axis=1)
    return np.ascontiguousarray(out_t.T[:N_NODES]).astype(np.float32)
